# revision 71
# baseline (speedup 1.0000x reference)
"""Trainium2 Bass kernel for nn_Attention_51634096833229.

Conv-projection attention block (CvT-style): depthwise 3x3 conv + BN on the
28x28 token image for each of q/k/v, linear qkv projections, 3-head attention
over 785 tokens (784 image + 1 cls), output projection.

Sharding: data-parallel over batch, B=32 -> 4 samples per core on 8 cores.

Design (TimelineSim ~166 us/core vs ~272 us for the v1 baseline):
  - Host pre-builds padded feature-major images: per sample-pair, 3 partition
    groups of [128, 901] bf16 rows = [cls | zero-padded 30x30 image].  Group 2
    packs both samples' channel tail (192=128+64) so conv work drops 25%.
    No PE transposes, no pad memsets/copies on device.
  - Depthwise conv+BN per (group, qkv-index) unit in three flavors spread
    across engines (GPSIMD cannot run scalar_tensor_tensor or touch PSUM):
      "v": DVE fused-MAC chain (f32 acc) - the dense but slow path.
      "P": Pool tensor_scalar scale-slabs + DVE bf16 tensor_tensor adds.
      "E": PE matmuls with host-built diag(w_tap) lhsT, rhs = shifted pad
           view, accumulated in PSUM (bank-aligned 504/280 column split);
           BN bias folded into the PSUM->SBUF tensor_scalar copy.  These
           run in the fill window while attention PSUM slots are free.
  - q,k: feature-major matmuls -> PSUM -> bf16 SBUF copies on ACT (queue
    depth 0, immune to the head-of-line blocking that DVE/Pool suffer).
  - v: token-major matmuls -> one strided ACT copy per token-block into a
    [t, 3*65] tile (per-head 64 cols + Pool-memset ones col for softmax
    denominators).
  - Attention: the pair's two samples are software-pipelined at token-block
    granularity (scores s0, scores s1, PV s0, PV s1) so ACT exp of one
    stream overlaps PE matmuls of the other.  exp on ACT with the scale
    folded in (|scores*scale|<3, no max subtraction).
  - Per-head normalize: PV copied out of PSUM first (frees the pv slot for
    the next head), then reciprocal (DVE) -> partition_broadcast (Pool) ->
    multiply (DVE) off the critical path.
  - Final projection per token-block with the bias folded in via an aT1
    ones-row against a bias row in wpa; the PSUM->SBUF evacuation is a plain
    copy (split DVE/ACT at the drain); outputs batched into one SBUF tile
    per sample -> 2 DMAs (avoids per-block DMA overhead).
  - The whole attention stream is emitted under tc.high_priority() so the
    compile-time list scheduler treats conv units as filler for idle slots.
"""

import os
import sys

sys.path.insert(0, "/opt/trn_rl_repo")

import numpy as np
import ml_dtypes

import concourse.bass as bass
import concourse.mybir as mybir
import concourse.tile as tile
from concourse import bacc
from concourse.bass_utils import run_bass_kernel_spmd

F32 = mybir.dt.float32
F32R = mybir.dt.float32r
BF16 = mybir.dt.bfloat16
AF = mybir.ActivationFunctionType
OP = mybir.AluOpType

B, T, C, CO, NH, D = 32, 785, 192, 192, 3, 64
HH = WW = 28
NCORES = 8
BPC = B // NCORES  # samples per core
NPAIR = BPC // 2
SCALE = float(CO) ** -0.5
BN_EPS = 1e-5
PADW = 901  # cls + 30*30 padded image

# token blocks of 128 along T
TBLK = [(i * 128, min(128, T - i * 128)) for i in range((T + 127) // 128)]
# N segments within 785 (psum bank = 512 f32)
NSEG = [(0, 512), (512, T - 512)]

VCOPY_ENG = os.environ.get("K_VCOPY", "act")
VONES_ENG = os.environ.get("K_VONES", "pool")
OBUF_ENG = os.environ.get("K_OBUF", "dve")
TTN_ENG = os.environ.get("K_TTN", "dve")

# conv unit order per pair: sample-0-relevant groups first (g0, g2), then g1.
# engine alternates D,P,...; 5 DVE + 4 Pool per pair -> 10/8 overall.
UNIT_ORDER = [(0, 0), (1, 0), (2, 0), (0, 1), (1, 1), (2, 1), (0, 2), (1, 2),
              (2, 2)]
# per-pair flavor lists: pair0 fill-optimized (fast DVE/PE), pair1 Pool-heavy
# (Pool is idle during pair0's attention).
UNIT_ENGINE_BY_PAIR = [
    ["E", "E", "P", "E", "P", "E", "E", "v", "v"],
    ["E", "E", "P", "E", "P", "E", "E", "v", "E"],
]


def _img3(ap, y=28, x=28):
    return ap.rearrange("p (y x) -> p y x", y=y, x=x)


def build_bass():
    nc = bacc.Bacc(None)
    xtg_d = nc.declare_dram_parameter("xtg", [NPAIR, 3, 128, PADW], BF16,
                                      isOutput=False)
    wc_d = nc.declare_dram_parameter("wc", [2, 128, 27], F32, isOutput=False)
    diag_d = nc.declare_dram_parameter("diag", [128, 54 * 128], BF16,
                                       isOutput=False)
    bnt_d = nc.declare_dram_parameter("bnt", [2, 128, 3], F32, isOutput=False)
    wq_d = nc.declare_dram_parameter("wq", [3, 128, CO], BF16, isOutput=False)
    wq2_d = nc.declare_dram_parameter("wq2", [3, 128, CO], BF16, isOutput=False)
    wpa_d = nc.declare_dram_parameter("wpa", [CO + 1, CO], BF16,
                                      isOutput=False)
    bias_d = nc.declare_dram_parameter("bias", [1, CO], F32, isOutput=False)
    out_d = nc.declare_dram_parameter("out", [BPC, T, CO], F32, isOutput=True)

    from contextlib import ExitStack
    with tile.TileContext(nc) as tc, ExitStack() as es:
        consts = es.enter_context(tc.tile_pool(name="consts", bufs=1))
        psA = es.enter_context(tc.tile_pool(name="psA", bufs=1, space="PSUM"))
        padp = es.enter_context(tc.tile_pool(name="pad", bufs=2))
        # (pad bufs=2 = one slot per pair; both pairs coexist)
        accp = es.enter_context(tc.tile_pool(name="acc", bufs=2))
        yp = es.enter_context(tc.tile_pool(name="y", bufs=2))
        qkp = es.enter_context(tc.tile_pool(name="qk", bufs=2))
        ep = es.enter_context(tc.tile_pool(name="E", bufs=14))
        vp = es.enter_context(tc.tile_pool(name="vsb", bufs=21))
        atp = es.enter_context(tc.tile_pool(name="aT", bufs=3))
        obp = es.enter_context(tc.tile_pool(name="ob", bufs=4))
        smallp = es.enter_context(tc.tile_pool(name="small", bufs=3))
        slabp = es.enter_context(tc.tile_pool(name="slab", bufs=3))

        # ---- per-pair state (loads emitted first so conv starts early) ----
        pad_tiles = {}
        y_tiles = {}

        def load_pair(p):
            for g in range(3):
                pt = padp.tile([128, PADW], BF16, tag=f"pad{g}", name=f"pad{g}")
                nc.sync.dma_start(pt[:], xtg_d[p, g])
                pad_tiles[(p, g)] = pt
                yt = yp.tile([128, 3 * T], BF16, tag=f"y{g}", name=f"y{g}")
                y_tiles[(p, g)] = yt

        # ---- constants (conv weights first, then pads, then the rest) ----
        wc_sb, bnt_sb = [], []
        for ci in range(2):
            t = consts.tile([128, 27], F32, tag=f"wc{ci}", name=f"wc{ci}")
            nc.sync.dma_start(t[:], wc_d[ci])
            wc_sb.append(t)
            t2 = consts.tile([128, 3], F32, tag=f"bnt{ci}", name=f"bnt{ci}")
            nc.sync.dma_start(t2[:], bnt_d[ci])
            bnt_sb.append(t2)
        diag_sb = consts.tile([128, 54 * 128], BF16, tag="diag", name="diag_sb")
        for ci in range(2):
            c0 = (ci * 27) * 128
            nc.sync.dma_start(diag_sb[:, c0:c0 + 9 * 128],
                              diag_d[:, c0:c0 + 9 * 128])
        load_pair(0)
        for i in range(1, 3):
            for ci in range(2):
                c0 = (ci * 27 + i * 9) * 128
                nc.sync.dma_start(diag_sb[:, c0:c0 + 9 * 128],
                                  diag_d[:, c0:c0 + 9 * 128])
        load_pair(1)
        wq_sb, wq2_sb = [], []
        for i in range(3):
            t = consts.tile([128, CO], BF16, tag=f"wq{i}", name=f"wq{i}")
            nc.sync.dma_start(t[:], wq_d[i])
            wq_sb.append(t)
            t2 = consts.tile([128, CO], BF16, tag=f"wq2{i}", name=f"wq2{i}")
            nc.sync.dma_start(t2[:], wq2_d[i])
            wq2_sb.append(t2)
        wpa0 = consts.tile([128, CO], BF16, tag="wpa0", name="wpa0")
        nc.sync.dma_start(wpa0[:], wpa_d[0:128, :])
        wpa1 = consts.tile([65, CO], BF16, tag="wpa1", name="wpa1")
        nc.sync.dma_start(wpa1[:], wpa_d[128:193, :])
        bias1 = consts.tile([1, CO], F32, tag="bias1", name="bias1")
        nc.sync.dma_start(bias1[:], bias_d[:])
        bias_bc = consts.tile([128, CO], F32, tag="biasbc", name="bias_bc")
        nc.gpsimd.partition_broadcast(bias_bc[:], bias1[0:1, :])

        def conv_unit(p, g, i, flavor):
            """depthwise conv + BN for (pair, group, qkv-index).

            flavor "v": DVE scalar_tensor_tensor chain (f32 acc).
            flavor "P": Pool tensor_scalar slabs + DVE bf16 tensor_tensor adds.
            flavor "E": PE diag-matmul accumulation in PSUM; BN bias applied
                        in the PSUM->SBUF tensor_scalar copy.
            """
            ci = 1 if g == 2 else 0
            pad = pad_tiles[(p, g)]
            y = y_tiles[(p, g)]
            pad3 = _img3(pad[:, 1:PADW], y=30, x=30)
            y3 = _img3(y[:, i * T + 1:(i + 1) * T])
            bntc = bnt_sb[ci][:, i:i + 1]

            def sh_(tap):
                dy, dx = tap // 3, tap % 3
                return pad3[:, dy:dy + 28, dx:dx + 28]

            def wc_(tap):
                return wc_sb[ci][:, i * 9 + tap:i * 9 + tap + 1]

            if flavor == "v":
                acc = accp.tile([128, 784], F32, tag=f"acc{g}", name=f"acc{g}")
                acc3 = _img3(acc[:])
                for tap in range(9):
                    if tap == 0:
                        nc.vector.tensor_scalar(acc3, sh_(0), wc_(0), bntc,
                                                OP.mult, OP.add)
                    elif tap < 8:
                        nc.vector.scalar_tensor_tensor(acc3, sh_(tap),
                                                       wc_(tap), acc3,
                                                       OP.mult, OP.add)
                    else:
                        nc.vector.scalar_tensor_tensor(y3, sh_(tap), wc_(tap),
                                                       acc3, OP.mult, OP.add)
                nc.gpsimd.tensor_copy(y[:, i * T:i * T + 1], pad[:, 0:1])
            elif flavor == "P":
                accb = slabp.tile([128, 784], BF16, tag="accP", name="accP")
                accb3 = _img3(accb[:])
                nc.gpsimd.tensor_scalar(accb3, sh_(0), wc_(0), bntc,
                                        OP.mult, OP.add)
                for tap in range(1, 9):
                    sl = slabp.tile([128, 784], BF16, tag=f"slab{tap % 2}",
                                    name="slab")
                    sl3 = _img3(sl[:])
                    nc.gpsimd.tensor_scalar(sl3, sh_(tap), wc_(tap), None,
                                            OP.mult)
                    if tap < 8:
                        nc.vector.tensor_tensor(accb3, accb3, sl3, OP.add)
                    else:
                        nc.vector.tensor_tensor(y3, accb3, sl3, OP.add)
                nc.gpsimd.tensor_copy(y[:, i * T:i * T + 1], pad[:, 0:1])
            elif flavor == "A":
                accb = slabp.tile([128, 784], BF16, tag="accP", name="accA")
                accb3 = _img3(accb[:])
                nc.vector.tensor_scalar(accb3, sh_(0), wc_(0), bntc,
                                        OP.mult, OP.add)
                for tap in range(1, 9):
                    sl = slabp.tile([128, 784], BF16, tag=f"slab{tap % 2}",
                                    name="slabA")
                    sl3 = _img3(sl[:])
                    nc.scalar.activation(sl3, sh_(tap), AF.Copy,
                                         scale=wc_(tap))
                    if tap < 8:
                        nc.vector.tensor_tensor(accb3, accb3, sl3, OP.add)
                    else:
                        nc.vector.tensor_tensor(y3, accb3, sl3, OP.add)
                nc.gpsimd.tensor_copy(y[:, i * T:i * T + 1], pad[:, 0:1])
            else:  # "E"
                ps = psA.tile([128, 792], F32, tag=f"pv{(g + i) % 2}", bufs=1,
                              name="cvps")
                segs = [(0, 0, 18), (512, 18, 28)]
                for tap in range(9):
                    didx = (ci * 27 + i * 9 + tap) * 128
                    dg = diag_sb[:, didx:didx + 128]
                    for (o0, y0, y1) in segs:
                        nc.tensor.matmul(
                            ps[0:128, o0:o0 + (y1 - y0) * 28],
                            dg, sh_(tap)[:, y0:y1, :],
                            start=(tap == 0), stop=(tap == 8))
                for (o0, y0, y1) in segs:
                    nc.vector.tensor_scalar(
                        _img3(y[:, i * T + 1 + y0 * 28:i * T + 1 + y1 * 28],
                              y=y1 - y0, x=28),
                        ps[0:128, o0:o0 + (y1 - y0) * 28].rearrange(
                            "p (y x) -> p y x", y=y1 - y0, x=28),
                        1.0, bntc, OP.mult, OP.add)
                nc.gpsimd.tensor_copy(y[:, i * T:i * T + 1], pad[:, 0:1])

        def y_ap(p, s, ci, i, c0, cn):
            """rhs/lhsT slice of y for sample s, channel-chunk ci, cols c0:c0+cn."""
            if ci == 0:
                return y_tiles[(p, s)][:, i * T + c0:i * T + c0 + cn]
            yt = y_tiles[(p, 2)]
            return yt[s * 64:(s + 1) * 64, i * T + c0:i * T + c0 + cn]

        def wq_ap(s, ci, i, o0, osz):
            if ci == 0:
                return wq_sb[i][:, o0:o0 + osz]
            return wq2_sb[i][s * 64:(s + 1) * 64, o0:o0 + osz]

        def qk_phase(p, s):
            """q,k feature-major projections -> qT,kT bf16 tiles [i][ob]."""
            qkT = []
            for i in range(2):
                row = []
                for ob, (o0, osz) in enumerate([(0, 128), (128, 64)]):
                    ps = psA.tile([128, T], F32, tag="ss", bufs=2, name="ssqk")
                    for (n0, nn) in NSEG:
                        for ci in range(2):
                            nc.tensor.matmul(
                                ps[0:osz, n0:n0 + nn],
                                wq_ap(s, ci, i, o0, osz),
                                y_ap(p, s, ci, i, n0, nn),
                                start=(ci == 0), stop=(ci == 1))
                    dst = qkp.tile([osz, T], BF16, tag=f"qk{i}{ob}{s}",
                                   name=f"qk{i}{ob}{s}")
                    nc.scalar.copy(dst[:], ps[0:osz, 0:T])
                    row.append(dst)
                qkT.append(row)
            return qkT

        def v_phase(p, s):
            """v token-major -> [t, 3*65] tiles with ones cols."""
            vs_tiles = []
            for tb, (t0, tn) in enumerate(TBLK):
                ps = psA.tile([128, T], F32, tag="ss", bufs=2, name="ssv")
                for ci in range(2):
                    nc.tensor.matmul(
                        ps[0:tn, 0:CO],
                        y_ap(p, s, ci, 2, t0, tn),
                        wq_ap(s, ci, 2, 0, CO),
                        start=(ci == 0), stop=(ci == 1))
                vs = vp.tile([128, 3 * 65], BF16, tag="vsb", name="vsb")
                vs3 = vs[:].rearrange("p (h c) -> p h c", h=3, c=65)
                if VCOPY_ENG == "act":
                    nc.scalar.copy(
                        vs3[0:tn, :, 0:64],
                        ps[0:tn, 0:CO].rearrange("p (h c) -> p h c", h=3, c=64))
                else:
                    nc.vector.tensor_copy(
                        vs3[0:tn, :, 0:64],
                        ps[0:tn, 0:CO].rearrange("p (h c) -> p h c", h=3, c=64))
                if VONES_ENG == "pool":
                    nc.gpsimd.memset(vs3[0:tn, :, 64:65], 1.0)
                elif VONES_ENG == "act":
                    nc.scalar.activation(
                        vs3[0:tn, :, 64:65],
                        bias_bc[0:tn, 0:3].rearrange("p (a b) -> p a b",
                                                     a=3, b=1),
                        AF.Identity, bias=1.0, scale=0.0)
                else:
                    nc.vector.memset(vs3[0:tn, :, 64:65], 1.0)
                vs_tiles.append(vs)
            return vs_tiles

        def head_rows(qkT, i, h):
            if h < 2:
                return qkT[i][0][h * 64:(h + 1) * 64, :]
            return qkT[i][1][0:64, :]

        def pair_attention(p, sprinkle):
            """Both samples of a pair, head chains interleaved at tblk level
            so PE/ACT ping-pong of one sample fills with the other's work."""
            qkTs = [qk_phase(p, 0), qk_phase(p, 1)]
            sprinkle()
            vss = [v_phase(p, 0), v_phase(p, 1)]
            sprinkle()
            aTs = []
            for s in range(2):
                a0 = atp.tile([128, T], BF16, tag=f"aT0{s}", name=f"aT0{s}")
                a1 = atp.tile([65, T], BF16, tag=f"aT1{s}", name=f"aT1{s}")
                nc.gpsimd.memset(a1[64:65, :], 1.0)
                aTs.append((a0, a1))
            for h in range(NH):
                pvs, es = {}, {}
                for s in range(2):
                    pvs[s] = psA.tile([128, T], F32, tag=f"pv{s}", bufs=1,
                                      name=f"pv{s}")
                for tb, (t0, tn) in enumerate(TBLK):
                    sss = {}
                    for s in range(2):
                        ss = psA.tile([128, T], F32, tag="ss", bufs=2,
                                      name="ss")
                        kh = head_rows(qkTs[s], 1, h)
                        qh = head_rows(qkTs[s], 0, h)
                        for (n0, nn) in NSEG:
                            nc.tensor.matmul(
                                ss[0:tn, n0:n0 + nn],
                                kh[:, t0:t0 + tn], qh[:, n0:n0 + nn],
                                start=True, stop=True)
                        sss[s] = ss
                    for s in range(2):
                        e = ep.tile([128, T], BF16, tag="E", name="E")
                        nc.scalar.activation(e[0:tn, 0:T], sss[s][0:tn, 0:T],
                                             AF.Exp, scale=SCALE)
                        es[s] = e
                    for s in range(2):
                        vs = vss[s][tb]
                        for (n0, nn) in NSEG:
                            nc.tensor.matmul(
                                pvs[s][0:65, n0:n0 + nn],
                                vs[0:tn, h * 65:(h + 1) * 65],
                                es[s][0:tn, n0:n0 + nn],
                                start=(tb == 0), stop=(tb == len(TBLK) - 1))
                for s in range(2):
                    # copy PV out of PSUM first so the pv slot frees for the
                    # next head; normalize runs off the critical path.
                    pv = pvs[s]
                    pvc = smallp.tile([65, T], F32, tag="pvc", bufs=3,
                                      name="pvc")
                    nc.vector.tensor_copy(pvc[:], pv[0:65, 0:T])
                    r = smallp.tile([1, T], F32, tag="r", name="r")
                    nc.vector.reciprocal(r[0:1, :], pvc[64:65, :])
                    rb = smallp.tile([64, T], F32, tag="rb", name="rb")
                    nc.gpsimd.partition_broadcast(rb[:], r[0:1, :])
                    dst = (aTs[s][0][h * 64:(h + 1) * 64, :] if h < 2
                           else aTs[s][1][0:64, :])
                    nc.vector.tensor_tensor(dst, pvc[0:64, :],
                                            rb[0:64, 0:T], OP.mult)
                sprinkle()

            # ---- final projection + bias + batched store ----
            for s in range(2):
                b = 2 * p + s
                obig = obp.tile([128, 6 * CO], F32, tag="obig", name="obig")
                otail = obp.tile([17, CO], F32, tag="otail", name="otail")
                for tb, (t0, tn) in enumerate(TBLK):
                    if p == NPAIR - 1:
                        fp = psA.tile([128, T], F32, tag="ss", bufs=2,
                                      name="fss")
                    else:
                        fp = psA.tile([128, T], F32, tag=f"pv{s}", bufs=1,
                                      name="fpv")
                    nc.tensor.matmul(fp[0:tn, 0:CO], aTs[s][0][:, t0:t0 + tn],
                                     wpa0[:], start=True, stop=False)
                    nc.tensor.matmul(fp[0:tn, 0:CO], aTs[s][1][:, t0:t0 + tn],
                                     wpa1[:], start=False, stop=True)
                    ob = (obig[:, tb * CO:(tb + 1) * CO] if tb < 6
                          else otail[:])
                    if p == NPAIR - 1 and tb % 2 == 1:
                        nc.scalar.copy(ob[0:tn, :], fp[0:tn, 0:CO])
                    else:
                        nc.vector.tensor_copy(ob[0:tn, :], fp[0:tn, 0:CO])
                nc.sync.dma_start(
                    out_d[b, 0:768, :].rearrange("(n p) c -> p n c", p=128),
                    obig[:].rearrange("p (n c) -> p n c", n=6, c=CO))
                nc.sync.dma_start(out_d[b, 768:785, :], otail[:])
                sprinkle()

        # ---- emission schedule ----
        # Conv units are emitted at natural (low) priority; the whole
        # attention stream gets high priority so the compile-time list
        # scheduler weaves conv ops into DVE/Pool idle slots as filler.
        for pp in range(NPAIR):
            for u, (g, i) in enumerate(UNIT_ORDER):
                conv_unit(pp, g, i, UNIT_ENGINE_BY_PAIR[pp][u])

        def no_sprinkle():
            pass

        with tc.high_priority(offset=1_000_000):
            pair_attention(0, no_sprinkle)
            pair_attention(1, no_sprinkle)

    if not nc.is_finalized():
        nc.finalize()
    return nc


_NC_CACHE = None


def _host_prep(x, conv_w, bn_scale, bn_bias, bn_mean, bn_var, w_qkv, w_proj,
               b_proj):
    """Builds per-core input maps (weights shared, xtg per core)."""
    s = bn_scale / np.sqrt(bn_var + BN_EPS)  # [3,C]
    wtap = (conv_w[:, :, 0, :, :].reshape(3, C, 9)
            * s[:, :, None]).astype(np.float32)  # [3,C,9]
    wc_full = np.ascontiguousarray(
        wtap.transpose(1, 0, 2).reshape(C, 27)).astype(np.float32)
    bnt_full = np.ascontiguousarray(
        (bn_bias - bn_mean * s).T).astype(np.float32)  # [C,3]
    wc_h = np.stack([wc_full[0:128],
                     np.concatenate([wc_full[128:192], wc_full[128:192]])])
    bnt_h = np.stack([bnt_full[0:128],
                      np.concatenate([bnt_full[128:192], bnt_full[128:192]])])
    wqT = np.ascontiguousarray(w_qkv.transpose(0, 2, 1))  # [3,C,CO]
    wq_h = wqT[:, 0:128, :].astype(ml_dtypes.bfloat16)
    wq2_h = np.concatenate([wqT[:, 128:192, :], wqT[:, 128:192, :]],
                           axis=1).astype(ml_dtypes.bfloat16)
    wpa_h = np.concatenate([w_proj.T, b_proj.reshape(1, CO)],
                           axis=0).astype(ml_dtypes.bfloat16)
    bias_h = b_proj.reshape(1, CO).astype(np.float32)

    # padded feature-major images per core / pair / group (bf16)
    xs = x.reshape(NCORES, BPC, T, C)
    xtg_all = np.zeros((NCORES, NPAIR, 3, 128, PADW), dtype=ml_dtypes.bfloat16)
    for c in range(NCORES):
        for p in range(NPAIR):
            rows = []
            for sidx in range(2):
                xi = xs[c, 2 * p + sidx]  # [T, C]
                row = np.zeros((C, PADW), dtype=np.float32)
                row[:, 0] = xi[0, :]
                pimg = row[:, 1:].reshape(C, 30, 30)
                pimg[:, 1:29, 1:29] = xi[1:, :].T.reshape(C, 28, 28)
                rows.append(row)
            xtg_all[c, p, 0] = rows[0][0:128]
            xtg_all[c, p, 1] = rows[1][0:128]
            xtg_all[c, p, 2, 0:64] = rows[0][128:192]
            xtg_all[c, p, 2, 64:128] = rows[1][128:192]

    diag_h = np.zeros((128, 54, 128), dtype=ml_dtypes.bfloat16)
    for ci in range(2):
        for col in range(27):
            w = wc_h[ci][:, col]
            idx = ci * 27 + col
            diag_h[np.arange(128), idx, np.arange(128)] = w.astype(
                ml_dtypes.bfloat16)
    diag_h = diag_h.reshape(128, 54 * 128)

    shared = {"wc": wc_h, "bnt": bnt_h, "wq": wq_h, "wq2": wq2_h,
              "wpa": wpa_h, "bias": bias_h, "diag": diag_h}
    return [dict(shared, xtg=np.ascontiguousarray(xtg_all[c]))
            for c in range(NCORES)]


def kernel(**inputs):
    global _NC_CACHE
    x = np.asarray(inputs["x"], dtype=np.float32)
    in_maps = _host_prep(
        x,
        np.asarray(inputs["conv_w"], dtype=np.float32),
        np.asarray(inputs["bn_scale"], dtype=np.float32),
        np.asarray(inputs["bn_bias"], dtype=np.float32),
        np.asarray(inputs["bn_mean"], dtype=np.float32),
        np.asarray(inputs["bn_var"], dtype=np.float32),
        np.asarray(inputs["w_qkv"], dtype=np.float32),
        np.asarray(inputs["w_proj"], dtype=np.float32),
        np.asarray(inputs["b_proj"], dtype=np.float32),
    )
    if _NC_CACHE is None:
        _NC_CACHE = build_bass()
    nc = _NC_CACHE
    res = run_bass_kernel_spmd(nc, in_maps, list(range(NCORES)), **RUN_KWARGS)
    global LAST_RESULTS
    LAST_RESULTS = res
    out = np.concatenate([np.asarray(r["out"]) for r in res.results], axis=0)
    return out.reshape(B, T, CO).astype(np.float32)


RUN_KWARGS = {}
LAST_RESULTS = None
